# revision 12
# baseline (speedup 1.0000x reference)
"""Tensor-parallel GQA attention prefill block for 8 Trainium2 NeuronCores.

Problem (hardcoded): x:[2,1024,4096] f32, 32 Q heads / 8 KV heads, head dim
128, RoPE at positions arange(1024), causal mask, KV-cache positions >=1024
masked out (cache starts zeroed), output projection Wo. The computation
reduces exactly to causal GQA attention + o_proj.

Sharding: tensor-parallel over heads. Core c owns Q heads 4c..4c+3 and KV
head c (Wq/Wk/Wv column shards), computes attention for its heads over all
tokens, then an AllToAll exchanges attention outputs so each core holds all
4096 features for a 128-token slice per batch; o_proj runs token-sharded
with the full (bf16) Wo; host concatenates the token slices.

v2 layout: attention scores are computed TRANSPOSED (S^T[k,q] per 128-k
block, lhsT = K chunk) so P^T feeds the PV matmul directly — no per-chunk
PE transposes. Softmax normalizer r = sum_k P^T is accumulated on DVE and
partition-reduced on the (otherwise idle) GpSimd engine, off the PE path.
DMA queues are split per engine so the AllToAll never blocks weight/x
prefetch, and o_proj runs batch-0's first column block during the tail
AllToAll.
"""
import sys

sys.path.insert(0, "/opt/trn_rl_repo")

import numpy as np
import ml_dtypes

import concourse.bass as bass
import concourse.tile as tile
from concourse import mybir
from concourse import bass_isa
from concourse.bass import ts
from concourse.bass_utils import run_bass_kernel_spmd

BF16 = mybir.dt.bfloat16
F32 = mybir.dt.float32
AF = mybir.ActivationFunctionType
OP = mybir.AluOpType
RED = bass_isa.ReduceOp

B, S, D = 2, 1024, 4096
H, KVH, HD = 32, 8, 128
NC = 8
QH = H // NC  # 4 q heads per core
THETA = 1000000.0
SC = 1.0 / float(np.sqrt(HD))
NFRONT = 10  # wo slabs held across both batches in o_proj dmq0

RG = [list(range(NC))]


def _build(split_for_walrus=True):
    nc = bass.Bass("TRN2", num_devices=NC)

    xT = nc.declare_dram_parameter("xT", [B, D, S], BF16, isOutput=False)
    wA = nc.declare_dram_parameter("wA", [D, 256], BF16, isOutput=False)
    wB = nc.declare_dram_parameter("wB", [D, 256], BF16, isOutput=False)
    wC = nc.declare_dram_parameter("wC", [D, 256], BF16, isOutput=False)
    wo = nc.declare_dram_parameter("wo", [D, D], BF16, isOutput=False)
    bias6 = nc.declare_dram_parameter("bias6", [6, 128], F32, isOutput=False)
    cosT = nc.declare_dram_parameter("cosT", [128, S], BF16, isOutput=False)
    sinT = nc.declare_dram_parameter("sinT", [128, S], BF16, isOutput=False)
    dmaskp = nc.declare_dram_parameter("dmaskp", [128, 128], BF16, isOutput=False)
    ident = nc.declare_dram_parameter("ident", [128, 128], BF16, isOutput=False)
    out = nc.declare_dram_parameter("out", [B, 128, D], F32, isOutput=True)

    from contextlib import ExitStack

    with ExitStack() as es:
        tc = es.enter_context(tile.TileContext(nc))
        cpool = es.enter_context(tc.tile_pool(name="consts", bufs=1))
        xcpool = es.enter_context(tc.tile_pool(name="xc", bufs=32))
        wpool = es.enter_context(tc.tile_pool(name="wslab", bufs=6))
        ropepool = es.enter_context(tc.tile_pool(name="rope", bufs=2))
        qrotpool = es.enter_context(tc.tile_pool(name="qrot", bufs=5))
        vtpool = es.enter_context(tc.tile_pool(name="vt", bufs=2))
        ppool = es.enter_context(tc.tile_pool(name="attn", bufs=8))
        rpool = es.enter_context(tc.tile_pool(name="rnorm", bufs=2))
        atpool = es.enter_context(tc.tile_pool(name="at", bufs=5))
        gpool = es.enter_context(tc.tile_pool(name="g", bufs=2))
        wofpool = es.enter_context(tc.tile_pool(name="wof", bufs=NFRONT))
        wospool = es.enter_context(tc.tile_pool(name="wos", bufs=8))
        ypool = es.enter_context(tc.tile_pool(name="ysb", bufs=2))
        psA = es.enter_context(tc.tile_pool(name="psA", bufs=3, space="PSUM"))
        psB = es.enter_context(tc.tile_pool(name="psB", bufs=2, space="PSUM"))
        dpool = es.enter_context(tc.tile_pool(name="dram", bufs=2, space="DRAM"))

        # constants (scalar queue; sync queue is reserved for x / wo)
        cos_sb = cpool.tile([128, S], BF16, tag="cos", name="cos")
        sin_sb = cpool.tile([128, S], BF16, tag="sin", name="sin")
        dmask_sb = cpool.tile([128, 128], BF16, tag="dmask", name="dmask")
        id_sb = cpool.tile([128, 128], BF16, tag="ident", name="ident")
        b_sb = cpool.tile([128, 6], F32, tag="bias", name="bias")
        ones_sb = cpool.tile([128, 1], BF16, tag="ones", name="ones")
        nc.scalar.dma_start(cos_sb[:], cosT[:])
        nc.scalar.dma_start(sin_sb[:], sinT[:])
        nc.scalar.dma_start(dmask_sb[:], dmaskp[:])
        nc.scalar.dma_start(id_sb[:], ident[:])
        nc.scalar.dma_start(b_sb[:], bias6[:].rearrange("i p -> p i"))
        nc.scalar.dma_start(ones_sb[:], dmaskp[:, 127:128])  # last triu col = all ones

        G = [None, None]

        for b in range(B):
            # ---- QKV projection + RoPE for batch b ----
            # x chunks on the sync queue, weight slabs on the scalar queue:
            # the two streams transfer in parallel and the first matmul can
            # start after ~one chunk of each.
            xc = []
            for k in range(32):
                t = xcpool.tile([128, S], BF16, tag="xc", name="xc")
                nc.sync.dma_start(t[:], xT[b, ts(k, 128), :])
                xc.append(t)

            rope_out = {}  # mg -> rotated tile
            v_sb = vtpool.tile([128, S], BF16, tag="v", name="v")
            for grp, wparam in ((0, wA), (1, wB), (2, wC)):
                pst = [
                    psA.tile([128, S], F32, tag="A", name="pst") for _ in range(2)
                ]
                for k in range(32):
                    slab = wpool.tile([128, 256], BF16, tag="wslab", name="wslab")
                    nc.scalar.dma_start(slab[:], wparam[ts(k, 128), :])
                    for m in range(2):
                        for n in range(2):
                            nc.tensor.matmul(
                                pst[m][:, ts(n, 512)],
                                slab[:, ts(m, 128)],
                                xc[k][:, ts(n, 512)],
                                start=(k == 0),
                                stop=(k == 31),
                            )
                for m in range(2):
                    mg = grp * 2 + m  # 0=Q0 1=K 2=V 3=Q1 4=Q2 5=Q3
                    if mg != 2:
                        q32 = ropepool.tile([128, S], F32, tag="q32", name="q32")
                        nc.scalar.activation(
                            q32[:], pst[m][:], AF.Identity,
                            bias=b_sb[:, mg : mg + 1],
                        )
                        sh = ropepool.tile([128, S], F32, tag="sh", name="sh")
                        nc.scalar.dma_start(sh[0:64, :], q32[64:128, :])
                        nc.scalar.dma_start(sh[64:128, :], q32[0:64, :])
                        nc.vector.tensor_mul(q32[:], q32[:], cos_sb[:])
                        nc.vector.tensor_mul(sh[:], sh[:], sin_sb[:])
                        rot = qrotpool.tile([128, S], BF16, tag="qrot", name="qrot")
                        nc.vector.tensor_add(rot[:], q32[:], sh[:])
                        rope_out[mg] = rot
                    else:
                        vt = vtpool.tile([128, S], BF16, tag="vt", name="vt")
                        nc.scalar.activation(
                            vt[:], pst[m][:], AF.Identity,
                            bias=b_sb[:, mg : mg + 1],
                        )
                        for j in range(8):
                            vp = psB.tile([128, 512], F32, tag="B", name="vp")
                            nc.tensor.matmul(
                                vp[:, 0:128], vt[:, ts(j, 128)], id_sb[:],
                                start=True, stop=True,
                            )
                            nc.vector.tensor_copy(v_sb[:, ts(j, 128)], vp[:, 0:128])

            K_t = rope_out[1]
            q_heads = [rope_out[0], rope_out[3], rope_out[4], rope_out[5]]

            # ---- attention, transposed-scores layout ----
            # Per head: S^T[k,q] blocks per 128-k chunk (lhsT = K chunk, rhs
            # = Q, causally trimmed widths), exp on ACT into P^T tiles, diag
            # 128x128 mask on DVE, normalizer r accumulated on DVE + one
            # GpSimd partition_all_reduce. O^T = V^T @ P^T accumulated over
            # k chunks, normalized into at[h] by one DVE multiply.
            at = [
                atpool.tile([128, S], BF16, tag="at", name="at")
                for _ in range(QH)
            ]

            for h in range(QH):
                Q_t = q_heads[h]
                racc = rpool.tile([128, S], F32, tag="racc", name="racc")
                Pts = []
                for kc in range(8):
                    qs = kc * 128
                    sp = psA.tile([128, S], F32, tag="A", name="sp")
                    if qs < 512:
                        nc.tensor.matmul(
                            sp[:, qs:512], K_t[:, ts(kc, 128)], Q_t[:, qs:512],
                            start=True, stop=True,
                        )
                        nc.tensor.matmul(
                            sp[:, 512:1024], K_t[:, ts(kc, 128)], Q_t[:, 512:1024],
                            start=True, stop=True,
                        )
                    else:
                        nc.tensor.matmul(
                            sp[:, qs:1024], K_t[:, ts(kc, 128)], Q_t[:, qs:1024],
                            start=True, stop=True,
                        )
                    P = ppool.tile([128, S], BF16, tag="psb", name="psb")
                    nc.scalar.activation(P[:, qs:1024], sp[:, qs:1024], AF.Exp, scale=SC)
                    # within-block causality: k_local <= q_local
                    nc.vector.tensor_mul(
                        P[:, qs : qs + 128], P[:, qs : qs + 128], dmask_sb[:]
                    )
                    if kc == 1:
                        nc.vector.tensor_copy(racc[:, 0:128], Pts[0][:, 0:128])
                        nc.vector.tensor_add(
                            racc[:, 128:1024], Pts[0][:, 128:1024], P[:, 128:1024]
                        )
                    elif kc > 1:
                        nc.vector.tensor_add(
                            racc[:, qs:1024], racc[:, qs:1024], P[:, qs:1024]
                        )
                    Pts.append(P)
                # collapse partitions: per-partition sums -> bf16 -> one
                # ones-matmul on PE -> recip -> DMA partition-broadcast
                racc_bf = rpool.tile([128, S], BF16, tag="raccbf", name="raccbf", bufs=1)
                nc.scalar.copy(racc_bf[:], racc[:])
                rps = [
                    psB.tile([128, 512], F32, tag="B", name="rp") for _ in range(2)
                ]
                for i in range(2):
                    nc.tensor.matmul(
                        rps[i][0:1, 0:512], ones_sb[:, 0:1],
                        racc_bf[:, ts(i, 512)],
                        start=True, stop=True,
                    )
                rrow = rpool.tile([1, S], BF16, tag="rrow", name="rrow", bufs=1)
                with nc.allow_low_precision(
                    reason="bf16 softmax normalizer matches baseline diag trick"
                ):
                    for i in range(2):
                        nc.vector.reciprocal(
                            rrow[0:1, ts(i, 512)], rps[i][0:1, 0:512]
                        )
                rrec = rpool.tile([128, S], BF16, tag="rrec", name="rrec")
                nc.gpsimd.dma_start(
                    rrec[:].unsqueeze(1),
                    rrow[:].unsqueeze(1).broadcast_to([1, 128, S]),
                )

                ot = psA.tile([128, S], F32, tag="A", name="ot")
                for kc in range(8):
                    qs = kc * 128
                    if qs < 512:
                        nc.tensor.matmul(
                            ot[:, qs:512], v_sb[:, ts(kc, 128)], Pts[kc][:, qs:512],
                            start=(kc == 0), stop=(kc == 3),
                        )
                        nc.tensor.matmul(
                            ot[:, 512:1024], v_sb[:, ts(kc, 128)],
                            Pts[kc][:, 512:1024],
                            start=(kc == 0), stop=(kc == 7),
                        )
                    else:
                        nc.tensor.matmul(
                            ot[:, qs:1024], v_sb[:, ts(kc, 128)], Pts[kc][:, qs:1024],
                            start=False, stop=(kc == 7),
                        )
                nc.vector.tensor_mul(at[h][:], ot[:], rrec[:])

            # ---- AllToAll: exchange head-shards for token-shards ----
            # staging DMAs ride the gpsimd queue and the gather the vector
            # queue so the collective wait never blocks x / weight prefetch.
            a2a_in = dpool.tile([NC, 512, 128], BF16, tag="a2ain", name="a2ain")
            for h in range(QH):
                nc.gpsimd.dma_start(
                    a2a_in[:].rearrange("d (hh p) t -> hh p d t", hh=QH)[h],
                    at[h][:].rearrange("p (d t) -> p d t", d=NC),
                )
            a2a_out = dpool.tile([NC, 512, 128], BF16, tag="a2aout", name="a2aout")
            nc.gpsimd.collective_compute(
                "AllToAll",
                OP.bypass,
                ins=[a2a_in[:].opt()],
                outs=[a2a_out[:].opt()],
                replica_groups=RG,
            )
            gt = gpool.tile([128, 4096], BF16, tag="g", name="g")
            nc.gpsimd.dma_start(
                gt[:].rearrange("p (fc t) -> p fc t", fc=32),
                a2a_out[:].rearrange("s (fl p) t -> p (s fl) t", p=128),
            )
            G[b] = gt

        # ---- token-sharded o_proj with full Wo ----
        # dmq0 runs batch 0 for all 32 fc chunks first: that PE work covers
        # the tail AllToAll latency for batch 1. The first NFRONT slabs are
        # held and reused for batch 1; the rest re-stream.
        def wo_dma(wt, fc, dmq):
            eng = nc.sync if fc % 2 == 0 else nc.scalar
            eng.dma_start(wt[:], wo[ts(fc, 128), dmq * 1024 : (dmq + 1) * 1024])

        wof = []
        for fc in range(NFRONT):
            wt = wofpool.tile([128, 1024], BF16, tag="wof", name="wof")
            wo_dma(wt, fc, 0)
            wof.append(wt)

        def emit_oproj(bi, dmq, yp, slabs):
            for fc in range(32):
                if slabs[fc] is None:
                    wt = wospool.tile([128, 1024], BF16, tag="wos", name="wos")
                    wo_dma(wt, fc, dmq)
                    slabs[fc] = wt
                for n in range(2):
                    nc.tensor.matmul(
                        yp[:, ts(n, 512)],
                        G[bi][:, ts(fc, 128)],
                        slabs[fc][:, ts(n, 512)],
                        start=(fc == 0), stop=(fc == 31),
                    )
            ys = ypool.tile([128, 1024], F32, tag="ysb", name="ys")
            nc.scalar.copy(ys[:], yp[:])
            nc.scalar.dma_start(
                out[bi, :, dmq * 1024 : (dmq + 1) * 1024], ys[:]
            )

        # dmq0: b0 fully, then b1 (front slabs reused, rest re-streamed)
        held = {fc: (wof[fc] if fc < NFRONT else None) for fc in range(32)}
        yp0 = psA.tile([128, 1024], F32, tag="A", name="yp0")
        emit_oproj(0, 0, yp0, dict(held))
        yp1 = psA.tile([128, 1024], F32, tag="A", name="yp1")
        emit_oproj(1, 0, yp1, dict(held))

        # dmq1..3: batches interleaved per fc chunk (single slab stream)
        for dmq in range(1, 4):
            yps = [
                psA.tile([128, 1024], F32, tag="A", name="yp")
                for _ in range(B)
            ]
            for fc in range(32):
                wt = wospool.tile([128, 1024], BF16, tag="wos", name="wos")
                wo_dma(wt, fc, dmq)
                for bi in range(B):
                    for n in range(2):
                        nc.tensor.matmul(
                            yps[bi][:, ts(n, 512)],
                            G[bi][:, ts(fc, 128)],
                            wt[:, ts(n, 512)],
                            start=(fc == 0), stop=(fc == 31),
                        )
            for bi in range(B):
                ys = ypool.tile([128, 1024], F32, tag="ysb", name="ys")
                nc.scalar.copy(ys[:], yps[bi][:])
                nc.scalar.dma_start(
                    out[bi, :, dmq * 1024 : (dmq + 1) * 1024], ys[:]
                )

    if split_for_walrus:
        _split_waits(nc, cap=1)
    return nc


def _split_waits(nc, cap=1):
    """This walrus build accepts at most one sync wait per instruction; hoist
    the excess onto same-engine NoOps inserted immediately before."""
    for fn in nc.m.functions:
        for bb in fn.blocks:
            new_insts = []
            for inst in bb.instructions:
                si = inst.sync_info
                if si is not None and si.on_wait and len(si.on_wait) > cap:
                    waits = list(si.on_wait)
                    head, rest = waits[: len(waits) - cap], waits[len(waits) - cap:]
                    for i in range(0, len(head), cap):
                        nop = mybir.InstNoOp(
                            name=f"{inst.name}-wsplit{i}", ins=[], outs=[]
                        )
                        nop.engine = inst.engine
                        nop.sync_info = mybir.SyncInfo(
                            on_wait=head[i : i + cap], on_update=[]
                        )
                        new_insts.append(nop)
                    inst.sync_info = mybir.SyncInfo(
                        on_wait=rest, on_update=list(si.on_update)
                    )
                new_insts.append(inst)
            bb.instructions = new_insts
    return nc


_NC_CACHE = None


def _get_nc():
    global _NC_CACHE
    if _NC_CACHE is None:
        _NC_CACHE = _build()
    return _NC_CACHE


def _prep_inputs(x, storage_idx, Wq, bq, Wk, bk, Wv, bv, Wo):
    bf = ml_dtypes.bfloat16
    xT = np.ascontiguousarray(
        np.asarray(x, np.float32).transpose(0, 2, 1)
    ).astype(bf)  # [B, D, S]
    wo_bf = np.ascontiguousarray(np.asarray(Wo, np.float32)).astype(bf)

    pos = np.asarray(storage_idx, np.int64).astype(np.float32)  # [S]
    inv = (1.0 / (THETA ** (np.arange(0, HD, 2, dtype=np.float32) / HD))).astype(
        np.float32
    )
    fr = pos[:, None] * inv[None, :]  # [S, 64]
    emb = np.concatenate([fr, fr], axis=1)  # [S, HD]
    cosT = np.ascontiguousarray(np.cos(emb).T.astype(np.float32)).astype(bf)  # [HD, S]
    sinT32 = np.ascontiguousarray(np.sin(emb).T).astype(np.float32)
    sinT32[0:64] *= -1.0
    sinT = sinT32.astype(bf)  # fold rotate_half sign

    # transposed-layout diagonal-block causal mask: P^T[k,q] valid iff k<=q
    dmask = np.triu(np.ones((128, 128), np.float32)).astype(bf)
    identity = np.eye(128, dtype=np.float32).astype(bf)

    in_maps = []
    for core in range(NC):
        q0 = core * 512
        kv = slice(core * 128, (core + 1) * 128)
        wAc = np.ascontiguousarray(
            np.concatenate([Wq[:, q0 : q0 + 128], Wk[:, kv]], axis=1)
        ).astype(bf)
        wBc = np.ascontiguousarray(
            np.concatenate([Wv[:, kv], Wq[:, q0 + 128 : q0 + 256]], axis=1)
        ).astype(bf)
        wCc = np.ascontiguousarray(Wq[:, q0 + 256 : q0 + 512]).astype(bf)
        bias6 = np.stack(
            [
                np.asarray(bq[q0 : q0 + 128], np.float32),
                np.asarray(bk[core * 128 : (core + 1) * 128], np.float32),
                np.asarray(bv[core * 128 : (core + 1) * 128], np.float32),
                np.asarray(bq[q0 + 128 : q0 + 256], np.float32),
                np.asarray(bq[q0 + 256 : q0 + 384], np.float32),
                np.asarray(bq[q0 + 384 : q0 + 512], np.float32),
            ]
        )  # [6, 128]
        in_maps.append(
            {
                "xT": xT,
                "wA": wAc,
                "wB": wBc,
                "wC": wCc,
                "wo": wo_bf,
                "bias6": np.ascontiguousarray(bias6),
                "cosT": cosT,
                "sinT": sinT,
                "dmaskp": dmask,
                "ident": identity,
            }
        )
    return in_maps


_LAST_RESULTS = None


def kernel(x, storage_idx, cache, mask, Wq, bq, Wk, bk, Wv, bv, Wo):
    """Full-input, full-output entry point. cache/mask are consumed implicitly:
    cache is zeros and positions >= S are causally masked, so the computation
    reduces to causal attention over the S prefill tokens."""
    global _LAST_RESULTS
    in_maps = _prep_inputs(x, storage_idx, Wq, bq, Wk, bk, Wv, bv, Wo)
    nc = _get_nc()
    res = run_bass_kernel_spmd(nc, in_maps, core_ids=list(range(NC)))
    _LAST_RESULTS = res
    full = np.empty((B, S, D), np.float32)
    for c in range(NC):
        o = res.results[c]["out"]  # [B, 128, D]
        for bi in range(B):
            full[bi, 128 * c : 128 * (c + 1), :] = o[bi]
    return full


# revision 15
# speedup vs baseline: 1.0570x; 1.0570x over previous
"""Tensor-parallel GQA attention prefill block for 8 Trainium2 NeuronCores.

Problem (hardcoded): x:[2,1024,4096] f32, 32 Q heads / 8 KV heads, head dim
128, RoPE at positions arange(1024), causal mask, KV-cache positions >=1024
masked out (cache starts zeroed), output projection Wo. The computation
reduces exactly to causal GQA attention + o_proj.

Sharding: tensor-parallel over heads. Core c owns Q heads 4c..4c+3 and KV
head c (Wq/Wk/Wv column shards), computes attention for its heads over all
tokens, then an AllToAll exchanges attention outputs so each core holds all
4096 features for a 128-token slice per batch; o_proj runs token-sharded
with the full (bf16) Wo; host concatenates the token slices.

v2 layout: attention scores are computed TRANSPOSED (S^T[k,q] per 128-k
block, lhsT = K chunk) so P^T feeds the PV matmul directly — no per-chunk
PE transposes. Softmax normalizer r = sum_k P^T is accumulated on DVE and
partition-reduced on the (otherwise idle) GpSimd engine, off the PE path.
DMA queues are split per engine so the AllToAll never blocks weight/x
prefetch, and o_proj runs batch-0's first column block during the tail
AllToAll.
"""
import sys

sys.path.insert(0, "/opt/trn_rl_repo")

import numpy as np
import ml_dtypes

import concourse.bass as bass
import concourse.tile as tile
from concourse import mybir
from concourse import bass_isa
from concourse.bass import ts
from concourse.bass_utils import run_bass_kernel_spmd

BF16 = mybir.dt.bfloat16
F32 = mybir.dt.float32
AF = mybir.ActivationFunctionType
OP = mybir.AluOpType
RED = bass_isa.ReduceOp

B, S, D = 2, 1024, 4096
H, KVH, HD = 32, 8, 128
NC = 8
QH = H // NC  # 4 q heads per core
THETA = 1000000.0
SC = 1.0 / float(np.sqrt(HD))
NFRONT = 10  # wo slabs held across both batches in o_proj dmq0

RG = [list(range(NC))]


def _build(split_for_walrus=True):
    nc = bass.Bass("TRN2", num_devices=NC)

    xT = nc.declare_dram_parameter("xT", [B, D, S], BF16, isOutput=False)
    wA = nc.declare_dram_parameter("wA", [D, 256], BF16, isOutput=False)
    wB = nc.declare_dram_parameter("wB", [D, 256], BF16, isOutput=False)
    wC = nc.declare_dram_parameter("wC", [D, 256], BF16, isOutput=False)
    wo = nc.declare_dram_parameter("wo", [D, D], BF16, isOutput=False)
    bias6 = nc.declare_dram_parameter("bias6", [6, 128], F32, isOutput=False)
    cosT = nc.declare_dram_parameter("cosT", [128, S], BF16, isOutput=False)
    sinT = nc.declare_dram_parameter("sinT", [128, S], BF16, isOutput=False)
    dmaskp = nc.declare_dram_parameter("dmaskp", [128, 128], BF16, isOutput=False)
    ident = nc.declare_dram_parameter("ident", [128, 128], BF16, isOutput=False)
    out = nc.declare_dram_parameter("out", [B, 128, D], F32, isOutput=True)

    from contextlib import ExitStack

    with ExitStack() as es:
        tc = es.enter_context(tile.TileContext(nc))
        cpool = es.enter_context(tc.tile_pool(name="consts", bufs=1))
        xcpool = es.enter_context(tc.tile_pool(name="xc", bufs=32))
        wpool = es.enter_context(tc.tile_pool(name="wslab", bufs=6))
        ropepool = es.enter_context(tc.tile_pool(name="rope", bufs=2))
        qrotpool = es.enter_context(tc.tile_pool(name="qrot", bufs=5))
        vtpool = es.enter_context(tc.tile_pool(name="vt", bufs=2))
        ppool = es.enter_context(tc.tile_pool(name="attn", bufs=8))
        rpool = es.enter_context(tc.tile_pool(name="rnorm", bufs=2))
        atpool = es.enter_context(tc.tile_pool(name="at", bufs=5))
        gpool = es.enter_context(tc.tile_pool(name="g", bufs=2))
        wofpool = es.enter_context(tc.tile_pool(name="wof", bufs=NFRONT))
        wospool = es.enter_context(tc.tile_pool(name="wos", bufs=8))
        ypool = es.enter_context(tc.tile_pool(name="ysb", bufs=2))
        psA = es.enter_context(tc.tile_pool(name="psA", bufs=3, space="PSUM"))
        psB = es.enter_context(tc.tile_pool(name="psB", bufs=2, space="PSUM"))
        dpool = es.enter_context(tc.tile_pool(name="dram", bufs=2, space="DRAM"))

        # constants (scalar queue; sync queue is reserved for x / wo)
        cos_sb = cpool.tile([128, S], BF16, tag="cos", name="cos")
        sin_sb = cpool.tile([128, S], BF16, tag="sin", name="sin")
        dmask_sb = cpool.tile([128, 128], BF16, tag="dmask", name="dmask")
        id_sb = cpool.tile([128, 128], BF16, tag="ident", name="ident")
        b_sb = cpool.tile([128, 6], F32, tag="bias", name="bias")
        ones_sb = cpool.tile([128, 1], BF16, tag="ones", name="ones")
        nc.sync.dma_start(cos_sb[:], cosT[:])
        nc.sync.dma_start(sin_sb[:], sinT[:])
        nc.sync.dma_start(dmask_sb[:], dmaskp[:])
        nc.sync.dma_start(id_sb[:], ident[:])
        nc.sync.dma_start(b_sb[:], bias6[:].rearrange("i p -> p i"))
        nc.sync.dma_start(ones_sb[:], dmaskp[:, 127:128])  # last triu col = all ones

        G = [None, None]

        for b in range(B):
            # ---- QKV projection + RoPE for batch b ----
            # x chunks on the sync queue, weight slabs on the scalar queue:
            # the two streams transfer in parallel and the first matmul can
            # start after ~one chunk of each.
            xc = []
            for k in range(32):
                t = xcpool.tile([128, S], BF16, tag="xc", name="xc")
                nc.sync.dma_start(t[:], xT[b, ts(k, 128), :])
                xc.append(t)

            rope_out = {}  # mg -> rotated tile
            v_sb = vtpool.tile([128, S], BF16, tag="v", name="v")
            for grp, wparam in ((0, wA), (1, wB), (2, wC)):
                pst = [
                    psA.tile([128, S], F32, tag="A", name="pst") for _ in range(2)
                ]
                for k in range(32):
                    slab = wpool.tile([128, 256], BF16, tag="wslab", name="wslab")
                    nc.sync.dma_start(slab[:], wparam[ts(k, 128), :])
                    for m in range(2):
                        for n in range(2):
                            nc.tensor.matmul(
                                pst[m][:, ts(n, 512)],
                                slab[:, ts(m, 128)],
                                xc[k][:, ts(n, 512)],
                                start=(k == 0),
                                stop=(k == 31),
                            )
                for m in range(2):
                    mg = grp * 2 + m  # 0=Q0 1=K 2=V 3=Q1 4=Q2 5=Q3
                    if mg != 2:
                        q32 = ropepool.tile([128, S], F32, tag="q32", name="q32")
                        nc.scalar.activation(
                            q32[:], pst[m][:], AF.Identity,
                            bias=b_sb[:, mg : mg + 1],
                        )
                        sh = ropepool.tile([128, S], F32, tag="sh", name="sh")
                        nc.scalar.dma_start(sh[0:64, :], q32[64:128, :])
                        nc.scalar.dma_start(sh[64:128, :], q32[0:64, :])
                        nc.vector.tensor_mul(q32[:], q32[:], cos_sb[:])
                        nc.vector.tensor_mul(sh[:], sh[:], sin_sb[:])
                        rot = qrotpool.tile([128, S], BF16, tag="qrot", name="qrot")
                        nc.vector.tensor_add(rot[:], q32[:], sh[:])
                        rope_out[mg] = rot
                    else:
                        vt = vtpool.tile([128, S], BF16, tag="vt", name="vt")
                        nc.scalar.activation(
                            vt[:], pst[m][:], AF.Identity,
                            bias=b_sb[:, mg : mg + 1],
                        )
                        for j in range(8):
                            vp = psB.tile([128, 512], F32, tag="B", name="vp")
                            nc.tensor.matmul(
                                vp[:, 0:128], vt[:, ts(j, 128)], id_sb[:],
                                start=True, stop=True,
                            )
                            nc.vector.tensor_copy(v_sb[:, ts(j, 128)], vp[:, 0:128])

            K_t = rope_out[1]
            q_heads = [rope_out[0], rope_out[3], rope_out[4], rope_out[5]]

            # ---- attention, transposed-scores layout ----
            # Per head: S^T[k,q] blocks per 128-k chunk (lhsT = K chunk, rhs
            # = Q, causally trimmed widths), exp on ACT into P^T tiles, diag
            # 128x128 mask on DVE, normalizer r accumulated on DVE + one
            # GpSimd partition_all_reduce. O^T = V^T @ P^T accumulated over
            # k chunks, normalized into at[h] by one DVE multiply.
            at = [
                atpool.tile([128, S], BF16, tag="at", name="at")
                for _ in range(QH)
            ]

            if b == 1:
                # prefetch o_proj dmq0 front slabs now: they transfer while
                # batch-1 attention computes, so the post-AllToAll o_proj
                # start is never slab-starved.
                wof = []
                for fc in range(NFRONT):
                    wt = wofpool.tile([128, 1024], BF16, tag="wof", name="wof")
                    nc.sync.dma_start(wt[:], wo[ts(fc, 128), 0:1024])
                    wof.append(wt)

            for h in range(QH):
                Q_t = q_heads[h]
                racc = rpool.tile([128, S], F32, tag="racc", name="racc")
                Pts = []
                for kc in range(8):
                    qs = kc * 128
                    sp = psA.tile([128, S], F32, tag="A", name="sp")
                    if qs < 512:
                        nc.tensor.matmul(
                            sp[:, qs:512], K_t[:, ts(kc, 128)], Q_t[:, qs:512],
                            start=True, stop=True,
                        )
                        nc.tensor.matmul(
                            sp[:, 512:1024], K_t[:, ts(kc, 128)], Q_t[:, 512:1024],
                            start=True, stop=True,
                        )
                    else:
                        nc.tensor.matmul(
                            sp[:, qs:1024], K_t[:, ts(kc, 128)], Q_t[:, qs:1024],
                            start=True, stop=True,
                        )
                    P = ppool.tile([128, S], BF16, tag="psb", name="psb")
                    nc.scalar.activation(P[:, qs:1024], sp[:, qs:1024], AF.Exp, scale=SC)
                    # within-block causality: k_local <= q_local
                    nc.gpsimd.tensor_mul(
                        P[:, qs : qs + 128], P[:, qs : qs + 128], dmask_sb[:]
                    )
                    if kc == 1:
                        nc.vector.tensor_copy(racc[:, 0:128], Pts[0][:, 0:128])
                        nc.vector.tensor_add(
                            racc[:, 128:1024], Pts[0][:, 128:1024], P[:, 128:1024]
                        )
                    elif kc > 1:
                        eng = nc.vector if kc < 4 else nc.gpsimd
                        eng.tensor_add(
                            racc[:, qs:1024], racc[:, qs:1024], P[:, qs:1024]
                        )
                    Pts.append(P)
                # collapse partitions: per-partition sums -> bf16 -> one
                # ones-matmul on PE -> recip -> DMA partition-broadcast
                racc_bf = rpool.tile([128, S], BF16, tag="raccbf", name="raccbf", bufs=1)
                nc.scalar.copy(racc_bf[:], racc[:])
                rps = [
                    psB.tile([128, 512], F32, tag="B", name="rp") for _ in range(2)
                ]
                for i in range(2):
                    nc.tensor.matmul(
                        rps[i][0:1, 0:512], ones_sb[:, 0:1],
                        racc_bf[:, ts(i, 512)],
                        start=True, stop=True,
                    )
                rrow = rpool.tile([1, S], BF16, tag="rrow", name="rrow", bufs=1)
                with nc.allow_low_precision(
                    reason="bf16 softmax normalizer matches baseline diag trick"
                ):
                    for i in range(2):
                        nc.vector.reciprocal(
                            rrow[0:1, ts(i, 512)], rps[i][0:1, 0:512]
                        )
                rrec = rpool.tile([128, S], BF16, tag="rrec", name="rrec")
                nc.gpsimd.dma_start(
                    rrec[:].unsqueeze(1),
                    rrow[:].unsqueeze(1).broadcast_to([1, 128, S]),
                )

                ot = psA.tile([128, S], F32, tag="A", name="ot")
                for kc in range(8):
                    qs = kc * 128
                    if qs < 512:
                        nc.tensor.matmul(
                            ot[:, qs:512], v_sb[:, ts(kc, 128)], Pts[kc][:, qs:512],
                            start=(kc == 0), stop=(kc == 3),
                        )
                        nc.tensor.matmul(
                            ot[:, 512:1024], v_sb[:, ts(kc, 128)],
                            Pts[kc][:, 512:1024],
                            start=(kc == 0), stop=(kc == 7),
                        )
                    else:
                        nc.tensor.matmul(
                            ot[:, qs:1024], v_sb[:, ts(kc, 128)], Pts[kc][:, qs:1024],
                            start=False, stop=(kc == 7),
                        )
                nc.vector.tensor_mul(at[h][:], ot[:], rrec[:])

            # ---- AllToAll: exchange head-shards for token-shards ----
            # staging DMAs ride the gpsimd queue and the gather the vector
            # queue so the collective wait never blocks x / weight prefetch.
            a2a_in = dpool.tile([NC, 512, 128], BF16, tag="a2ain", name="a2ain")
            for h in range(QH):
                nc.gpsimd.dma_start(
                    a2a_in[:].rearrange("d (hh p) t -> hh p d t", hh=QH)[h],
                    at[h][:].rearrange("p (d t) -> p d t", d=NC),
                )
            a2a_out = dpool.tile([NC, 512, 128], BF16, tag="a2aout", name="a2aout")
            nc.gpsimd.collective_compute(
                "AllToAll",
                OP.bypass,
                ins=[a2a_in[:].opt()],
                outs=[a2a_out[:].opt()],
                replica_groups=RG,
            )
            gt = gpool.tile([128, 4096], BF16, tag="g", name="g")
            nc.gpsimd.dma_start(
                gt[:].rearrange("p (fc t) -> p fc t", fc=32),
                a2a_out[:].rearrange("s (fl p) t -> p (s fl) t", p=128),
            )
            G[b] = gt

        # ---- token-sharded o_proj with full Wo ----
        # dmq0 runs batch 0 for all 32 fc chunks first: that PE work covers
        # the tail AllToAll latency for batch 1. The first NFRONT slabs are
        # held and reused for batch 1; the rest re-stream.
        def wo_dma(wt, fc, dmq):
            nc.sync.dma_start(wt[:], wo[ts(fc, 128), dmq * 1024 : (dmq + 1) * 1024])

        def emit_oproj(bi, dmq, yp, slabs):
            for fc in range(32):
                if slabs[fc] is None:
                    wt = wospool.tile([128, 1024], BF16, tag="wos", name="wos")
                    wo_dma(wt, fc, dmq)
                    slabs[fc] = wt
                for n in range(2):
                    nc.tensor.matmul(
                        yp[:, ts(n, 512)],
                        G[bi][:, ts(fc, 128)],
                        slabs[fc][:, ts(n, 512)],
                        start=(fc == 0), stop=(fc == 31),
                    )
            ys = ypool.tile([128, 1024], F32, tag="ysb", name="ys")
            nc.scalar.copy(ys[:], yp[:])
            nc.scalar.dma_start(
                out[bi, :, dmq * 1024 : (dmq + 1) * 1024], ys[:]
            )

        # dmq0: b0 fully, then b1 (front slabs reused, rest re-streamed)
        held = {fc: (wof[fc] if fc < NFRONT else None) for fc in range(32)}
        yp0 = psA.tile([128, 1024], F32, tag="A", name="yp0")
        emit_oproj(0, 0, yp0, dict(held))
        yp1 = psA.tile([128, 1024], F32, tag="A", name="yp1")
        emit_oproj(1, 0, yp1, dict(held))

        # dmq1..3: batches interleaved per fc chunk (single slab stream)
        for dmq in range(1, 4):
            yps = [
                psA.tile([128, 1024], F32, tag="A", name="yp")
                for _ in range(B)
            ]
            for fc in range(32):
                wt = wospool.tile([128, 1024], BF16, tag="wos", name="wos")
                wo_dma(wt, fc, dmq)
                for bi in range(B):
                    for n in range(2):
                        nc.tensor.matmul(
                            yps[bi][:, ts(n, 512)],
                            G[bi][:, ts(fc, 128)],
                            wt[:, ts(n, 512)],
                            start=(fc == 0), stop=(fc == 31),
                        )
            for bi in range(B):
                ys = ypool.tile([128, 1024], F32, tag="ysb", name="ys")
                nc.scalar.copy(ys[:], yps[bi][:])
                nc.scalar.dma_start(
                    out[bi, :, dmq * 1024 : (dmq + 1) * 1024], ys[:]
                )

    if split_for_walrus:
        _split_waits(nc, cap=1)
    return nc


def _split_waits(nc, cap=1):
    """This walrus build accepts at most one sync wait per instruction; hoist
    the excess onto same-engine NoOps inserted immediately before."""
    for fn in nc.m.functions:
        for bb in fn.blocks:
            new_insts = []
            for inst in bb.instructions:
                si = inst.sync_info
                if si is not None and si.on_wait and len(si.on_wait) > cap:
                    waits = list(si.on_wait)
                    head, rest = waits[: len(waits) - cap], waits[len(waits) - cap:]
                    for i in range(0, len(head), cap):
                        nop = mybir.InstNoOp(
                            name=f"{inst.name}-wsplit{i}", ins=[], outs=[]
                        )
                        nop.engine = inst.engine
                        nop.sync_info = mybir.SyncInfo(
                            on_wait=head[i : i + cap], on_update=[]
                        )
                        new_insts.append(nop)
                    inst.sync_info = mybir.SyncInfo(
                        on_wait=rest, on_update=list(si.on_update)
                    )
                new_insts.append(inst)
            bb.instructions = new_insts
    return nc


_NC_CACHE = None


def _get_nc():
    global _NC_CACHE
    if _NC_CACHE is None:
        _NC_CACHE = _build()
    return _NC_CACHE


def _prep_inputs(x, storage_idx, Wq, bq, Wk, bk, Wv, bv, Wo):
    bf = ml_dtypes.bfloat16
    xT = np.ascontiguousarray(
        np.asarray(x, np.float32).transpose(0, 2, 1)
    ).astype(bf)  # [B, D, S]
    wo_bf = np.ascontiguousarray(np.asarray(Wo, np.float32)).astype(bf)

    pos = np.asarray(storage_idx, np.int64).astype(np.float32)  # [S]
    inv = (1.0 / (THETA ** (np.arange(0, HD, 2, dtype=np.float32) / HD))).astype(
        np.float32
    )
    fr = pos[:, None] * inv[None, :]  # [S, 64]
    emb = np.concatenate([fr, fr], axis=1)  # [S, HD]
    cosT = np.ascontiguousarray(np.cos(emb).T.astype(np.float32)).astype(bf)  # [HD, S]
    sinT32 = np.ascontiguousarray(np.sin(emb).T).astype(np.float32)
    sinT32[0:64] *= -1.0
    sinT = sinT32.astype(bf)  # fold rotate_half sign

    # transposed-layout diagonal-block causal mask: P^T[k,q] valid iff k<=q
    dmask = np.triu(np.ones((128, 128), np.float32)).astype(bf)
    identity = np.eye(128, dtype=np.float32).astype(bf)

    in_maps = []
    for core in range(NC):
        q0 = core * 512
        kv = slice(core * 128, (core + 1) * 128)
        wAc = np.ascontiguousarray(
            np.concatenate([Wq[:, q0 : q0 + 128], Wk[:, kv]], axis=1)
        ).astype(bf)
        wBc = np.ascontiguousarray(
            np.concatenate([Wv[:, kv], Wq[:, q0 + 128 : q0 + 256]], axis=1)
        ).astype(bf)
        wCc = np.ascontiguousarray(Wq[:, q0 + 256 : q0 + 512]).astype(bf)
        bias6 = np.stack(
            [
                np.asarray(bq[q0 : q0 + 128], np.float32),
                np.asarray(bk[core * 128 : (core + 1) * 128], np.float32),
                np.asarray(bv[core * 128 : (core + 1) * 128], np.float32),
                np.asarray(bq[q0 + 128 : q0 + 256], np.float32),
                np.asarray(bq[q0 + 256 : q0 + 384], np.float32),
                np.asarray(bq[q0 + 384 : q0 + 512], np.float32),
            ]
        )  # [6, 128]
        in_maps.append(
            {
                "xT": xT,
                "wA": wAc,
                "wB": wBc,
                "wC": wCc,
                "wo": wo_bf,
                "bias6": np.ascontiguousarray(bias6),
                "cosT": cosT,
                "sinT": sinT,
                "dmaskp": dmask,
                "ident": identity,
            }
        )
    return in_maps


_LAST_RESULTS = None


def kernel(x, storage_idx, cache, mask, Wq, bq, Wk, bk, Wv, bv, Wo):
    """Full-input, full-output entry point. cache/mask are consumed implicitly:
    cache is zeros and positions >= S are causally masked, so the computation
    reduces to causal attention over the S prefill tokens."""
    global _LAST_RESULTS
    in_maps = _prep_inputs(x, storage_idx, Wq, bq, Wk, bk, Wv, bv, Wo)
    nc = _get_nc()
    res = run_bass_kernel_spmd(nc, in_maps, core_ids=list(range(NC)))
    _LAST_RESULTS = res
    full = np.empty((B, S, D), np.float32)
    for c in range(NC):
        o = res.results[c]["out"]  # [B, 128, D]
        for bi in range(B):
            full[bi, 128 * c : 128 * (c + 1), :] = o[bi]
    return full
